# revision 1
# baseline (speedup 1.0000x reference)
"""Trainium2 Bass kernel for ContinuousLSTMLayer (RK4 ODE-LSTM).

Contract: kernel(**inputs) takes FULL unsharded inputs, returns FULL output
[B, S, H].  Internally: pure data parallelism over 8 NeuronCores (batch dim),
state kept transposed [H, B_local] on-chip, gates computed via tanh-only
activations with weight prescaling, RK4 stage matmuls as PSUM delta
accumulations.
"""

import sys

sys.path.insert(0, "/opt/trn_rl_repo")

import numpy as np

B, S, F, H = 256, 512, 64, 128
NCORES = 8
BL = B // NCORES  # 32 batch per core
PAD = 8  # extra zero steps so prefetches past the end stay in bounds
SP = S + PAD
MAX_DT = 1.0
ODE_STEPS = 4

_GATES = ["f", "i", "o", "g"]  # column order in the fused gate tile
_GSCALE = {"f": 0.5, "i": 0.5, "o": 0.5, "g": 1.0}  # tanh-only trick


def _host_prep(x, time_diffs, Ws, bs):
    """Build per-core input dicts (numpy only)."""
    f4 = np.float32
    # Fused weights [128, 512] / [65, 512], gate order f,i,o,g.
    Wh = np.concatenate([Ws[g][F:] * _GSCALE[g] for g in _GATES], axis=1).astype(f4)
    Wx = np.concatenate(
        [np.vstack([Ws[g][:F], bs[g][None, :]]) * _GSCALE[g] for g in _GATES], axis=1
    ).astype(f4)
    # Scan weights: per (pair, j) free index = pair*4 + j, d0 = [0, .5, 2, 2]
    swts = np.tile(np.array([0.0, 0.5, 2.0, 2.0], f4), 2 * BL)[None, :].repeat(128, 0)
    swts = np.ascontiguousarray(swts)  # [128, 256]

    in_maps = []
    for c in range(NCORES):
        sl = slice(BL * c, BL * (c + 1))
        xc = x[sl]  # [BL, S, F]
        # xT_aug [65, SP*BL]: [f, t*BL + b] = x[b, t, f]; row 64 = 1.0
        xt = np.zeros((F + 1, SP * BL), f4)
        xt[:F, : S * BL] = xc.transpose(2, 1, 0).reshape(F, S * BL)
        xt[F, : S * BL] = 1.0
        # dt2rep [(SP)*128, 128]: per step rows t*128..t*128+127 all equal:
        # cols 0:64 = 0.25*sd tiled twice, cols 64:128 = 0.5*sd tiled twice
        sd = (np.minimum(time_diffs[sl], MAX_DT) / ODE_STEPS).T.astype(f4)  # [S, BL]
        row = np.zeros((SP, 128), f4)
        row[:S, 0:BL] = 0.25 * sd
        row[:S, BL : 2 * BL] = 0.25 * sd
        row[:S, 2 * BL : 3 * BL] = 0.5 * sd
        row[:S, 3 * BL : 4 * BL] = 0.5 * sd
        dt2 = np.ascontiguousarray(
            np.broadcast_to(row[:, None, :], (SP, 128, 128)).reshape(SP * 128, 128)
        )
        in_maps.append(
            {
                "Wh": Wh,
                "Wx": Wx,
                "swts": swts,
                "xT": xt,
                "dt2": dt2,
            }
        )
    return in_maps


def _build(nc, n_steps=S):
    import concourse.mybir as mybir
    from concourse.tile import TileContext
    from concourse.bass import ds
    from contextlib import ExitStack

    f32 = mybir.dt.float32
    Alu = mybir.AluOpType
    Act = mybir.ActivationFunctionType

    Wh_d = nc.dram_tensor("Wh", [128, 512], f32, kind="ExternalInput").ap()
    Wx_d = nc.dram_tensor("Wx", [F + 1, 512], f32, kind="ExternalInput").ap()
    swts_d = nc.dram_tensor("swts", [128, 8 * BL], f32, kind="ExternalInput").ap()
    xT_d = nc.dram_tensor("xT", [F + 1, SP * BL], f32, kind="ExternalInput").ap()
    dt2_d = nc.dram_tensor("dt2", [SP * 128, 128], f32, kind="ExternalInput").ap()
    out_d = nc.dram_tensor("hT", [n_steps * 128, BL], f32, kind="ExternalOutput").ap()

    NSLOT = 8  # steps per For_i body

    with TileContext(nc) as tc, ExitStack() as ctx:
        const = ctx.enter_context(tc.tile_pool(name="const", bufs=1))
        Wh = const.tile([128, 512], f32)
        Wx = const.tile([F + 1, 512], f32)
        swts = const.tile([128, 8 * BL], f32)
        nc.sync.dma_start(Wh[:], Wh_d[:])
        nc.sync.dma_start(Wx[:], Wx_d[:])
        nc.sync.dma_start(swts[:], swts_d[:])

        st = ctx.enter_context(tc.tile_pool(name="state", bufs=1))
        base = [st.tile([128, 2 * BL], f32, name=f"base{p}") for p in range(2)]
        stile = [st.tile([128, 2 * BL], f32, name=f"s{p}") for p in range(2)]
        kdall = st.tile([128, 8 * BL], f32)  # [128, pair*4 + j]
        xts = [st.tile([F + 1, BL], f32, name=f"xt{k}") for k in range(NSLOT)]
        dts = [st.tile([128, 128], f32, name=f"dt{k}") for k in range(NSLOT)]

        work = ctx.enter_context(tc.tile_pool(name="work", bufs=2))
        pspool = ctx.enter_context(tc.tile_pool(name="ps", bufs=2, space="PSUM"))

        nc.vector.memset(base[0][:], 0.0)

        kd4 = kdall[:].rearrange("p (n j) -> p n j", j=4)  # [128, 64, 4]

        def load_slot(k, toff):
            """toff: runtime or python int giving the step index."""
            if isinstance(toff, int):
                nc.sync.dma_start(xts[k][:], xT_d[:, toff * BL : (toff + 1) * BL])
                nc.sync.dma_start(
                    dts[k][:], dt2_d[toff * 128 : (toff + 1) * 128, :]
                )
            else:
                nc.sync.dma_start(xts[k][:], xT_d[:, ds(toff * BL, BL)])
                nc.sync.dma_start(dts[k][:], dt2_d[ds(toff * 128, 128), :])

        def one_step(slot, trow):
            """trow: runtime value (step index) for the output DMA row offset."""
            xt, dtt = xts[slot], dts[slot]
            for m in range(ODE_STEPS):
                bread = base[m % 2]
                bwrite = base[(m + 1) % 2]
                ps = pspool.tile([128, 128], f32, tag="pre")
                # ---- base group: pre = Wh.T @ h + Wx.T @ x_aug (per gate cols)
                for g in range(4):
                    nc.tensor.matmul(
                        ps[:, g * BL : (g + 1) * BL],
                        Wh[:, g * 128 : (g + 1) * 128],
                        bread[:, BL : 2 * BL],
                        start=(g == 0),
                        stop=True,
                        skip_group_check=True,
                    )
                    nc.tensor.matmul(
                        ps[:, g * BL : (g + 1) * BL],
                        Wx[:, g * 128 : (g + 1) * 128],
                        xt[:],
                        start=False,
                        stop=True,
                        skip_group_check=True,
                    )
                for j in range(4):
                    if j == 0:
                        s = bread
                    else:
                        s = stile[(j + 1) % 2]
                        # stage matmul: pre += Wh.T @ (kd_{j-1} - kd_{j-2})_h
                        if j == 1:
                            rhs = kd4[:, BL : 2 * BL, 0]
                        else:
                            rhs = work.tile([128, BL], f32, tag="mmrhs")
                            nc.vector.tensor_tensor(
                                rhs[:],
                                kd4[:, BL : 2 * BL, j - 1],
                                kd4[:, BL : 2 * BL, j - 2],
                                Alu.subtract,
                            )
                            rhs = rhs[:]
                        for g in range(4):
                            nc.tensor.matmul(
                                ps[:, g * BL : (g + 1) * BL],
                                Wh[:, g * 128 : (g + 1) * 128],
                                rhs,
                                start=False,
                                stop=True,
                                skip_group_check=True,
                            )
                    # ---- elementwise stage
                    T = work.tile([128, 5 * BL], f32, tag="T")
                    nc.scalar.activation(T[:, 0 : 4 * BL], ps[:, :], Act.Tanh)
                    nc.scalar.activation(
                        T[:, 4 * BL : 5 * BL], s[:, 0:BL], Act.Tanh
                    )
                    P = work.tile([128, 2 * BL], f32, tag="P")
                    # P = (T[i,o] + 1) * [Tg, tanh(c)] = [2ig | 2o*tanh(c)]
                    nc.vector.scalar_tensor_tensor(
                        P[:], T[:, BL : 3 * BL], 1.0, T[:, 3 * BL : 5 * BL],
                        Alu.add, Alu.mult,
                    )
                    Fq = work.tile([128, BL], f32, tag="Fq")
                    # Fq = (Tf - 1) * c = 2(f-1)c
                    nc.vector.scalar_tensor_tensor(
                        Fq[:], T[:, 0:BL], 1.0, s[:, 0:BL], Alu.subtract, Alu.mult
                    )
                    k2 = work.tile([128, 2 * BL], f32, tag="k2")
                    nc.vector.tensor_tensor(k2[:, 0:BL], P[:, 0:BL], Fq[:], Alu.add)
                    # k2h = -2*h + 2*o*tanh(c)
                    nc.vector.scalar_tensor_tensor(
                        k2[:, BL : 2 * BL], s[:, BL : 2 * BL], -2.0,
                        P[:, BL : 2 * BL], Alu.mult, Alu.add,
                    )
                    # kd_j = dt_j * k2  (dt cols: 0:64 = dt/16, 64:128 = dt/8)
                    dslice = dtt[:, 0 : 2 * BL] if j < 2 else dtt[:, 2 * BL : 4 * BL]
                    nc.vector.tensor_tensor(kd4[:, :, j], k2[:], dslice, Alu.mult)
                    if j < 3:
                        nc.vector.tensor_tensor(
                            stile[j % 2][:], bread[:], kd4[:, :, j], Alu.add
                        )
                # ---- RK4 combine: scan gives S = 2kd0+4kd1+2kd2+kd3 at j=3 cols
                sc = work.tile([128, 8 * BL], f32, tag="sc")
                nc.vector.tensor_tensor_scan(
                    sc[:], swts[:], kdall[:], 0.0, Alu.mult, Alu.add
                )
                nc.vector.scalar_tensor_tensor(
                    bwrite[:],
                    sc[:].rearrange("p (n j) -> p n j", j=4)[:, :, 3],
                    1.0 / 6.0,
                    bread[:],
                    Alu.mult,
                    Alu.add,
                )
            # write h half of the final state for this step
            if isinstance(trow, int):
                nc.sync.dma_start(
                    out_d[trow * 128 : (trow + 1) * 128, :], base[0][:, BL : 2 * BL]
                )
            else:
                nc.sync.dma_start(
                    out_d[ds(trow * 128, 128), :], base[0][:, BL : 2 * BL]
                )

        # prologue: slots 0..3 <- steps 0..3
        for k in range(4):
            load_slot(k, k)

        if n_steps <= NSLOT:
            # static tiny version (for simulation/debug)
            for k in range(4, NSLOT):
                load_slot(k, min(k, SP - 1))
            for t in range(n_steps):
                one_step(t % NSLOT, t)
        else:
            assert n_steps % NSLOT == 0
            with tc.For_i(0, n_steps, NSLOT) as i:
                for k in range(4, NSLOT):
                    load_slot(k, i + k)
                for j in range(4):
                    one_step(j, i + j)
                for k in range(4):
                    load_slot(k, i + NSLOT + k)
                for j in range(4, NSLOT):
                    one_step(j, i + j)
    nc.finalize()
    return nc


_NC_CACHE = {}


def _get_nc(n_steps=S):
    if n_steps not in _NC_CACHE:
        import concourse.bacc as bacc

        nc = bacc.Bacc(
            "TRN2", target_bir_lowering=False, debug=False, num_devices=NCORES
        )
        _NC_CACHE[n_steps] = _build(nc, n_steps)
    return _NC_CACHE[n_steps]


def kernel(x, time_diffs, W_i, b_i, W_f, b_f, W_o, b_o, W_g, b_g):
    from concourse.bass_utils import run_bass_kernel_spmd

    x = np.asarray(x, np.float32)
    time_diffs = np.asarray(time_diffs, np.float32)
    Ws = {"i": W_i, "f": W_f, "o": W_o, "g": W_g}
    bs = {"i": b_i, "f": b_f, "o": b_o, "g": b_g}
    Ws = {k: np.asarray(v, np.float32) for k, v in Ws.items()}
    bs = {k: np.asarray(v, np.float32) for k, v in bs.items()}

    in_maps = _host_prep(x, time_diffs, Ws, bs)
    nc = _get_nc(S)
    res = run_bass_kernel_spmd(nc, in_maps, list(range(NCORES)))
    globals()["_last_results"] = res
    out = np.empty((B, S, H), np.float32)
    for c in range(NCORES):
        hT = res.results[c]["hT"].reshape(S, 128, BL)  # [t, h, b]
        out[BL * c : BL * (c + 1)] = hT.transpose(2, 0, 1)
    return out


if __name__ == "__main__":
    # quick build-only check
    n = int(sys.argv[1]) if len(sys.argv) > 1 else 8
    nc = _get_nc(n)
    print("built ok, instructions:", sum(len(bb.instructions) for bb in nc.m.functions[0].blocks))



# revision 9
# speedup vs baseline: 9.3378x; 9.3378x over previous
"""Trainium2 Bass kernel for ContinuousLSTMLayer (RK4 ODE-LSTM).

Contract: kernel(**inputs) takes FULL unsharded inputs, returns FULL output
[B, S, H].  Internally: pure data parallelism over 8 NeuronCores (batch dim),
state kept transposed [H, B_local] on-chip, gates computed via tanh-only
activations with weight prescaling, RK4 stage matmuls as PSUM delta
accumulations.

v1 transfer optimizations vs the original baseline:
  - the per-step dt broadcast tile is built ON-CHIP from a tiny [SP,128]
    "srow" input via a K=1 matmul against a ones vector (replaces the
    272MB host-expanded dt2 input).
  - custom PJRT exec path with a cached jit and device-resident zero
    output buffers (the stock path re-uploads ~67MB of zeros per call).
"""

import sys

sys.path.insert(0, "/opt/trn_rl_repo")

import numpy as np

B, S, F, H = 256, 512, 64, 128
NCORES = 8
BL = B // NCORES  # 32 batch per core
PAD = 8  # extra zero steps so prefetches past the end stay in bounds
SP = S + PAD
MAX_DT = 1.0
ODE_STEPS = 4

_GATES = ["f", "i", "o", "g"]  # column order in the fused gate tile
_GSCALE = {"f": 0.5, "i": 0.5, "o": 0.5, "g": 1.0}  # tanh-only trick


def _host_prep(x, time_diffs, Ws, bs):
    """Build per-core input dicts (numpy only)."""
    f4 = np.float32
    # Fused weights [128, 512] / [65, 512], gate order f,i,o,g.
    Wh = np.concatenate([Ws[g][F:] * _GSCALE[g] for g in _GATES], axis=1).astype(f4)
    Wx = np.concatenate(
        [np.vstack([Ws[g][:F], bs[g][None, :]]) * _GSCALE[g] for g in _GATES], axis=1
    ).astype(f4)
    # Scan weights: per (pair, j) free index = pair*4 + j, d0 = [0, .5, 2, 2]
    swts = np.tile(np.array([0.0, 0.5, 2.0, 2.0], f4), 2 * BL)[None, :].repeat(128, 0)
    swts = np.ascontiguousarray(swts)  # [128, 256]

    in_maps = []
    for c in range(NCORES):
        sl = slice(BL * c, BL * (c + 1))
        xc = x[sl]  # [BL, S, F]
        # xT_aug [65, SP*BL]: [f, t*BL + b] = x[b, t, f]; row 64 = 1.0
        xt = np.zeros((F + 1, SP * BL), f4)
        xt[:F, : S * BL] = xc.transpose(2, 1, 0).reshape(F, S * BL)
        xt[F, : S * BL] = 1.0
        # srow [SP, 128]: per-step dt row, broadcast on-chip to [128, 128]:
        # cols 0:64 = 0.25*sd tiled twice, cols 64:128 = 0.5*sd tiled twice
        sd = (np.minimum(time_diffs[sl], MAX_DT) / ODE_STEPS).T.astype(f4)  # [S, BL]
        srow = np.zeros((SP, 128), f4)
        srow[:S, 0:BL] = 0.25 * sd
        srow[:S, BL : 2 * BL] = 0.25 * sd
        srow[:S, 2 * BL : 3 * BL] = 0.5 * sd
        srow[:S, 3 * BL : 4 * BL] = 0.5 * sd
        srow = np.ascontiguousarray(srow.reshape(1, SP * 128))
        in_maps.append(
            {
                "Wh": Wh,
                "Wx": Wx,
                "swts": swts,
                "xT": xt,
                "srow": srow,
            }
        )
    return in_maps


def _build(nc, n_steps=S):
    import concourse.mybir as mybir
    from concourse.tile import TileContext
    from concourse.bass import ds
    from contextlib import ExitStack

    f32 = mybir.dt.float32
    Alu = mybir.AluOpType
    Act = mybir.ActivationFunctionType

    Wh_d = nc.dram_tensor("Wh", [128, 512], f32, kind="ExternalInput").ap()
    Wx_d = nc.dram_tensor("Wx", [F + 1, 512], f32, kind="ExternalInput").ap()
    swts_d = nc.dram_tensor("swts", [128, 8 * BL], f32, kind="ExternalInput").ap()
    xT_d = nc.dram_tensor("xT", [F + 1, SP * BL], f32, kind="ExternalInput").ap()
    srow_d = nc.dram_tensor("srow", [1, SP * 128], f32, kind="ExternalInput").ap()
    out_d = nc.dram_tensor("hT", [n_steps * 128, BL], f32, kind="ExternalOutput").ap()

    NSLOT = 8  # steps per For_i body

    with TileContext(nc) as tc, ExitStack() as ctx:
        const = ctx.enter_context(tc.tile_pool(name="const", bufs=1))
        Wh = const.tile([128, 512], f32)
        Wx = const.tile([F + 1, 512], f32)
        swts = const.tile([128, 8 * BL], f32)
        ones = const.tile([1, 128], f32)
        nc.sync.dma_start(Wh[:], Wh_d[:])
        nc.sync.dma_start(Wx[:], Wx_d[:])
        nc.sync.dma_start(swts[:], swts_d[:])
        nc.vector.memset(ones[:], 1.0)

        st = ctx.enter_context(tc.tile_pool(name="state", bufs=1))
        base = [st.tile([128, 2 * BL], f32, name=f"base{p}") for p in range(2)]
        stile = [st.tile([128, 2 * BL], f32, name=f"s{p}") for p in range(2)]
        kdall = st.tile([128, 8 * BL], f32)  # [128, pair*4 + j]
        # half-body staging: xts[h] covers 4 steps of xT, srows[h] 4 dt rows
        xts = [st.tile([F + 1, 4 * BL], f32, name=f"xt{h}") for h in range(2)]
        srows = [st.tile([1, 4 * 128], f32, name=f"sr{h}") for h in range(2)]

        work = ctx.enter_context(tc.tile_pool(name="work", bufs=2))
        pspool = ctx.enter_context(tc.tile_pool(name="ps", bufs=2, space="PSUM"))
        dtpool = ctx.enter_context(tc.tile_pool(name="dt", bufs=2, space="PSUM"))

        nc.vector.memset(base[0][:], 0.0)

        kd4 = kdall[:].rearrange("p (n j) -> p n j", j=4)  # [128, 64, 4]

        def load_half(h, toff):
            """Load 4 steps of x columns + dt rows starting at step `toff`."""
            if isinstance(toff, int):
                nc.sync.dma_start(xts[h][:], xT_d[:, toff * BL : (toff + 4) * BL])
                nc.sync.dma_start(
                    srows[h][:], srow_d[:, toff * 128 : (toff + 4) * 128]
                )
            else:
                nc.sync.dma_start(xts[h][:], xT_d[:, ds(toff * BL, 4 * BL)])
                nc.sync.dma_start(srows[h][:], srow_d[:, ds(toff * 128, 4 * 128)])

        def one_step(h, k, trow):
            """h: half (0/1), k: step-in-half (0..3), trow: runtime step idx."""
            xt = xts[h][:, k * BL : (k + 1) * BL]
            # broadcast dt row -> [128, 128] in PSUM via K=1 matmul with ones
            dtt = dtpool.tile([128, 128], f32, tag="dtt")
            nc.tensor.matmul(
                dtt[:],
                ones[:],
                srows[h][:, k * 128 : (k + 1) * 128],
                start=True,
                stop=True,
            )
            for m in range(ODE_STEPS):
                bread = base[m % 2]
                bwrite = base[(m + 1) % 2]
                ps = pspool.tile([128, 128], f32, tag="pre")
                # ---- base group: pre = Wh.T @ h + Wx.T @ x_aug (per gate cols)
                for g in range(4):
                    nc.tensor.matmul(
                        ps[:, g * BL : (g + 1) * BL],
                        Wh[:, g * 128 : (g + 1) * 128],
                        bread[:, BL : 2 * BL],
                        start=(g == 0),
                        stop=True,
                        skip_group_check=True,
                    )
                    nc.tensor.matmul(
                        ps[:, g * BL : (g + 1) * BL],
                        Wx[:, g * 128 : (g + 1) * 128],
                        xt,
                        start=False,
                        stop=True,
                        skip_group_check=True,
                    )
                for j in range(4):
                    if j == 0:
                        s = bread
                    else:
                        s = stile[(j + 1) % 2]
                        # stage matmul: pre += Wh.T @ (kd_{j-1} - kd_{j-2})_h
                        if j == 1:
                            rhs = kd4[:, BL : 2 * BL, 0]
                        else:
                            rhs = work.tile([128, BL], f32, tag="mmrhs")
                            nc.vector.tensor_tensor(
                                rhs[:],
                                kd4[:, BL : 2 * BL, j - 1],
                                kd4[:, BL : 2 * BL, j - 2],
                                Alu.subtract,
                            )
                            rhs = rhs[:]
                        for g in range(4):
                            nc.tensor.matmul(
                                ps[:, g * BL : (g + 1) * BL],
                                Wh[:, g * 128 : (g + 1) * 128],
                                rhs,
                                start=False,
                                stop=True,
                                skip_group_check=True,
                            )
                    # ---- elementwise stage
                    T = work.tile([128, 5 * BL], f32, tag="T")
                    nc.scalar.activation(T[:, 0 : 4 * BL], ps[:, :], Act.Tanh)
                    nc.scalar.activation(
                        T[:, 4 * BL : 5 * BL], s[:, 0:BL], Act.Tanh
                    )
                    P = work.tile([128, 2 * BL], f32, tag="P")
                    # P = (T[i,o] + 1) * [Tg, tanh(c)] = [2ig | 2o*tanh(c)]
                    nc.vector.scalar_tensor_tensor(
                        P[:], T[:, BL : 3 * BL], 1.0, T[:, 3 * BL : 5 * BL],
                        Alu.add, Alu.mult,
                    )
                    Fq = work.tile([128, BL], f32, tag="Fq")
                    # Fq = (Tf - 1) * c = 2(f-1)c
                    nc.vector.scalar_tensor_tensor(
                        Fq[:], T[:, 0:BL], 1.0, s[:, 0:BL], Alu.subtract, Alu.mult
                    )
                    k2 = work.tile([128, 2 * BL], f32, tag="k2")
                    nc.vector.tensor_tensor(k2[:, 0:BL], P[:, 0:BL], Fq[:], Alu.add)
                    # k2h = -2*h + 2*o*tanh(c)
                    nc.vector.scalar_tensor_tensor(
                        k2[:, BL : 2 * BL], s[:, BL : 2 * BL], -2.0,
                        P[:, BL : 2 * BL], Alu.mult, Alu.add,
                    )
                    # kd_j = dt_j * k2  (dtt cols: 0:64 = dt/16, 64:128 = dt/8)
                    dslice = dtt[:, 0 : 2 * BL] if j < 2 else dtt[:, 2 * BL : 4 * BL]
                    nc.vector.tensor_tensor(kd4[:, :, j], k2[:], dslice, Alu.mult)
                    if j < 3:
                        nc.vector.tensor_tensor(
                            stile[j % 2][:], bread[:], kd4[:, :, j], Alu.add
                        )
                # ---- RK4 combine: scan gives S = 2kd0+4kd1+2kd2+kd3 at j=3 cols
                sc = work.tile([128, 8 * BL], f32, tag="sc")
                nc.vector.tensor_tensor_scan(
                    sc[:], swts[:], kdall[:], 0.0, Alu.mult, Alu.add
                )
                nc.vector.scalar_tensor_tensor(
                    bwrite[:],
                    sc[:].rearrange("p (n j) -> p n j", j=4)[:, :, 3],
                    1.0 / 6.0,
                    bread[:],
                    Alu.mult,
                    Alu.add,
                )
            # write h half of the final state for this step
            if isinstance(trow, int):
                nc.sync.dma_start(
                    out_d[trow * 128 : (trow + 1) * 128, :], base[0][:, BL : 2 * BL]
                )
            else:
                nc.sync.dma_start(
                    out_d[ds(trow * 128, 128), :], base[0][:, BL : 2 * BL]
                )

        # prologue: half 0 <- steps 0..3
        load_half(0, 0)

        if n_steps <= NSLOT:
            # static tiny version (for simulation/debug)
            load_half(1, 4)
            for t in range(n_steps):
                one_step(t // 4, t % 4, t)
        else:
            assert n_steps % NSLOT == 0
            with tc.For_i(0, n_steps, NSLOT) as i:
                load_half(1, i + 4)
                for k in range(4):
                    one_step(0, k, i + k)
                load_half(0, i + NSLOT)
                for k in range(4):
                    one_step(1, k, i + 4 + k)
    nc.finalize()
    return nc


_NC_CACHE = {}


def _get_nc(n_steps=S):
    if n_steps not in _NC_CACHE:
        import concourse.bacc as bacc

        nc = bacc.Bacc(
            "TRN2", target_bir_lowering=False, debug=False, num_devices=NCORES
        )
        _NC_CACHE[n_steps] = _build(nc, n_steps)
    return _NC_CACHE[n_steps]


# ---------------------------------------------------------------------------
# Custom exec path: like bass2jax.run_bass_via_pjrt but with a cached jit and
# device-resident zero output buffers (no 67MB host->device zeros per call).
# ---------------------------------------------------------------------------

_FN_CACHE = {}


def _get_runner(nc):
    key = id(nc)
    if key in _FN_CACHE:
        return _FN_CACHE[key]

    import jax
    import jax.numpy as jnp
    from jax.sharding import Mesh, PartitionSpec, NamedSharding
    from jax.experimental.shard_map import shard_map
    import concourse.mybir as mybir
    from concourse.bass2jax import (
        _bass_exec_p,
        install_neuronx_cc_hook,
        partition_id_tensor,
    )

    install_neuronx_cc_hook()

    partition_name = (
        nc.partition_id_tensor.name if nc.partition_id_tensor else None
    )
    in_names, out_names, out_avals = [], [], []
    for alloc in nc.m.functions[0].allocations:
        if not isinstance(alloc, mybir.MemoryLocationSet):
            continue
        name = alloc.memorylocations[0].name
        if alloc.kind == "ExternalInput":
            if name != partition_name:
                in_names.append(name)
        elif alloc.kind == "ExternalOutput":
            shape = tuple(alloc.tensor_shape)
            dtype = mybir.dt.np(alloc.dtype)
            out_names.append(name)
            out_avals.append(jax.core.ShapedArray(shape, dtype))
    n_params = len(in_names)
    all_names = in_names + out_names
    if partition_name is not None:
        all_names = all_names + [partition_name]

    def _body(*args):
        operands = list(args)
        operands.append(partition_id_tensor())
        outs = _bass_exec_p.bind(
            *operands,
            out_avals=tuple(out_avals),
            in_names=tuple(all_names),
            out_names=tuple(out_names),
            lowering_input_output_aliases=(),
            sim_require_finite=True,
            sim_require_nnan=True,
            nc=nc,
        )
        return tuple(outs)

    devices = jax.devices()[:NCORES]
    mesh = Mesh(np.asarray(devices), ("core",))
    nin = n_params + len(out_names)
    fn = jax.jit(
        shard_map(
            _body,
            mesh=mesh,
            in_specs=(PartitionSpec("core"),) * nin,
            out_specs=(PartitionSpec("core"),) * len(out_names),
            check_rep=False,
        ),
        keep_unused=True,
    )
    sharding = NamedSharding(mesh, PartitionSpec("core"))
    # device-resident zero stand-ins for the output buffers (never donated,
    # never mutated -- the kernel writes every output element)
    zeros = [
        jax.jit(
            lambda a=a: jnp.zeros((NCORES * a.shape[0], *a.shape[1:]), a.dtype),
            out_shardings=sharding,
        )()
        for a in out_avals
    ]
    runner = (fn, in_names, out_names, out_avals, zeros, sharding)
    _FN_CACHE[key] = runner
    return runner


class _Results:
    """Minimal stand-in for BassKernelResults (test.py reads .results/.exec_time_ns)."""

    def __init__(self, results):
        self.results = results
        self.exec_time_ns = None


def _run(nc, in_maps):
    fn, in_names, out_names, out_avals, zeros, _ = _get_runner(nc)
    concat_in = [
        np.concatenate([in_maps[c][n] for c in range(NCORES)], axis=0)
        for n in in_names
    ]
    out_arrs = fn(*concat_in, *zeros)
    results = [
        {
            name: np.asarray(out_arrs[i]).reshape(NCORES, *out_avals[i].shape)[c]
            for i, name in enumerate(out_names)
        }
        for c in range(NCORES)
    ]
    return _Results(results)


def kernel(x, time_diffs, W_i, b_i, W_f, b_f, W_o, b_o, W_g, b_g):
    x = np.asarray(x, np.float32)
    time_diffs = np.asarray(time_diffs, np.float32)
    Ws = {"i": W_i, "f": W_f, "o": W_o, "g": W_g}
    bs = {"i": b_i, "f": b_f, "o": b_o, "g": b_g}
    Ws = {k: np.asarray(v, np.float32) for k, v in Ws.items()}
    bs = {k: np.asarray(v, np.float32) for k, v in bs.items()}

    in_maps = _host_prep(x, time_diffs, Ws, bs)
    nc = _get_nc(S)
    res = _run(nc, in_maps)
    globals()["_last_results"] = res
    out = np.empty((B, S, H), np.float32)
    for c in range(NCORES):
        hT = res.results[c]["hT"].reshape(S, 128, BL)  # [t, h, b]
        out[BL * c : BL * (c + 1)] = hT.transpose(2, 0, 1)
    return out


def _bench_device(iters=3):
    """Time the jitted exec with pre-staged device inputs (upload excluded)."""
    import time
    import jax

    names = ["x", "time_diffs"] + [
        f"{p}_{g}" for g in "ifog" for p in ("W", "b")
    ]
    ins = {n: np.load(f"/root/problem/work/in_{n}.npy") for n in names}
    Ws = {k: ins[f"W_{k}"] for k in "ifog"}
    bs = {k: ins[f"b_{k}"] for k in "ifog"}
    in_maps = _host_prep(ins["x"], ins["time_diffs"], Ws, bs)
    nc = _get_nc(S)
    fn, in_names, out_names, out_avals, zeros, sharding = _get_runner(nc)
    concat_in = [
        np.concatenate([in_maps[c][n] for c in range(NCORES)], axis=0)
        for n in in_names
    ]
    dev_in = [jax.device_put(a, sharding) for a in concat_in]
    for a in dev_in:
        a.block_until_ready()
    outs = fn(*dev_in, *zeros)  # warm (compile already cached)
    [o.block_until_ready() for o in outs]
    times = []
    for _ in range(iters):
        t0 = time.time()
        outs = fn(*dev_in, *zeros)
        [o.block_until_ready() for o in outs]
        times.append(time.time() - t0)
    return min(times)


if __name__ == "__main__":
    # quick build-only check
    n = int(sys.argv[1]) if len(sys.argv) > 1 else 8
    nc = _get_nc(n)
    print(
        "built ok, instructions:",
        sum(len(bb.instructions) for bb in nc.m.functions[0].blocks),
    )


# revision 16
# speedup vs baseline: 18.3961x; 1.9701x over previous
"""Trainium2 Bass kernel for ContinuousLSTMLayer (RK4 ODE-LSTM).

Contract: kernel(**inputs) takes FULL unsharded inputs, returns FULL output
[B, S, H].  Internally: pure data parallelism over 8 NeuronCores (batch dim),
state kept transposed [H, B_local] on-chip, gates computed via tanh-only
activations with weight prescaling, RK4 stage matmuls as PSUM delta
accumulations.

v1 transfer optimizations vs the original baseline:
  - the per-step dt broadcast tile is built ON-CHIP from a tiny [SP,128]
    "srow" input via a K=1 matmul against a ones vector (replaces the
    272MB host-expanded dt2 input).
  - custom PJRT exec path with a cached jit and device-resident zero
    output buffers (the stock path re-uploads ~67MB of zeros per call).
"""

import sys

sys.path.insert(0, "/opt/trn_rl_repo")

import numpy as np

B, S, F, H = 256, 512, 64, 128
NCORES = 8
BL = B // NCORES  # 32 batch per core
PAD = 8  # extra zero steps so prefetches past the end stay in bounds
SP = S + PAD
MAX_DT = 1.0
ODE_STEPS = 4

_GATES = ["f", "i", "o", "g"]  # column order in the fused gate tile
_GSCALE = {"f": 0.5, "i": 0.5, "o": 0.5, "g": 1.0}  # tanh-only trick


def _host_prep(x, time_diffs, Ws, bs):
    """Build per-core input dicts (numpy only)."""
    f4 = np.float32
    try:
        import ml_dtypes

        bf16 = ml_dtypes.bfloat16
    except ImportError:  # pragma: no cover
        bf16 = np.float32
    # Fused weights [128, 512] / [65, 512], gate order f,i,o,g.
    Wh = np.concatenate([Ws[g][F:] * _GSCALE[g] for g in _GATES], axis=1).astype(f4)
    Wx = np.concatenate(
        [np.vstack([Ws[g][:F], bs[g][None, :]]) * _GSCALE[g] for g in _GATES], axis=1
    ).astype(bf16)
    # Scan weights: per (pair, j) free index = pair*4 + j, d0 = [0, .5, 2, 2]
    swts = np.tile(np.array([0.0, 0.5, 2.0, 2.0], f4), 2 * BL)[None, :].repeat(128, 0)
    swts = np.ascontiguousarray(swts)  # [128, 256]

    in_maps = []
    for c in range(NCORES):
        sl = slice(BL * c, BL * (c + 1))
        xc = x[sl]  # [BL, S, F]
        # xT_aug [65, SP*BL]: [f, t*BL + b] = x[b, t, f]; row 64 = 1.0
        xt = np.zeros((F + 1, SP * BL), bf16)
        xt[:F, : S * BL] = xc.transpose(2, 1, 0).reshape(F, S * BL).astype(bf16)
        xt[F, : S * BL] = 1.0
        # srow [SP, 128]: per-step dt row, broadcast on-chip to [128, 128]:
        # cols 0:64 = 0.25*sd tiled twice, cols 64:128 = 0.5*sd tiled twice
        sd = (np.minimum(time_diffs[sl], MAX_DT) / ODE_STEPS).T.astype(f4)  # [S, BL]
        srow = np.zeros((SP, 128), f4)
        srow[:S, 0:BL] = 0.25 * sd
        srow[:S, BL : 2 * BL] = 0.25 * sd
        srow[:S, 2 * BL : 3 * BL] = 0.5 * sd
        srow[:S, 3 * BL : 4 * BL] = 0.5 * sd
        srow = np.ascontiguousarray(srow.reshape(1, SP * 128))
        in_maps.append(
            {
                "Wh": Wh,
                "Wx": Wx,
                "swts": swts,
                "xT": xt,
                "srow": srow,
            }
        )
    return in_maps


def _build(nc, n_steps=S):
    import concourse.mybir as mybir
    from concourse.tile import TileContext
    from concourse.bass import ds
    from contextlib import ExitStack

    f32 = mybir.dt.float32
    bf16 = mybir.dt.bfloat16
    Alu = mybir.AluOpType
    Act = mybir.ActivationFunctionType

    Wh_d = nc.dram_tensor("Wh", [128, 512], f32, kind="ExternalInput").ap()
    Wx_d = nc.dram_tensor("Wx", [F + 1, 512], bf16, kind="ExternalInput").ap()
    swts_d = nc.dram_tensor("swts", [128, 8 * BL], f32, kind="ExternalInput").ap()
    xT_d = nc.dram_tensor("xT", [F + 1, SP * BL], bf16, kind="ExternalInput").ap()
    srow_d = nc.dram_tensor("srow", [1, SP * 128], f32, kind="ExternalInput").ap()
    out_d = nc.dram_tensor(
        "hT", [n_steps * 128, BL], bf16, kind="ExternalOutput"
    ).ap()

    NSLOT = 8  # steps per For_i body

    with TileContext(nc) as tc, ExitStack() as ctx:
        const = ctx.enter_context(tc.tile_pool(name="const", bufs=1))
        Wh = const.tile([128, 512], f32)
        Wx = const.tile([F + 1, 512], bf16)
        swts = const.tile([128, 8 * BL], f32)
        ones = const.tile([1, 128], f32)
        nc.sync.dma_start(Wh[:], Wh_d[:])
        nc.sync.dma_start(Wx[:], Wx_d[:])
        nc.sync.dma_start(swts[:], swts_d[:])
        nc.vector.memset(ones[:], 1.0)

        st = ctx.enter_context(tc.tile_pool(name="state", bufs=1))
        base = [st.tile([128, 2 * BL], f32, name=f"base{p}") for p in range(2)]
        stile = [st.tile([128, 2 * BL], f32, name=f"s{p}") for p in range(2)]
        kdall = st.tile([128, 8 * BL], f32)  # [128, pair*4 + j]
        # half-body staging: xts[h] covers 4 steps of xT, srows[h] 4 dt rows
        xts = [st.tile([F + 1, 4 * BL], bf16, name=f"xt{h}") for h in range(2)]
        srows = [st.tile([1, 4 * 128], f32, name=f"sr{h}") for h in range(2)]

        work = ctx.enter_context(tc.tile_pool(name="work", bufs=2))
        pspool = ctx.enter_context(tc.tile_pool(name="ps", bufs=2, space="PSUM"))
        dtpool = ctx.enter_context(tc.tile_pool(name="dt", bufs=2, space="PSUM"))

        nc.vector.memset(base[0][:], 0.0)

        kd4 = kdall[:].rearrange("p (n j) -> p n j", j=4)  # [128, 64, 4]

        def load_half(h, toff):
            """Load 4 steps of x columns + dt rows starting at step `toff`."""
            if isinstance(toff, int):
                nc.sync.dma_start(xts[h][:], xT_d[:, toff * BL : (toff + 4) * BL])
                nc.sync.dma_start(
                    srows[h][:], srow_d[:, toff * 128 : (toff + 4) * 128]
                )
            else:
                nc.sync.dma_start(xts[h][:], xT_d[:, ds(toff * BL, 4 * BL)])
                nc.sync.dma_start(srows[h][:], srow_d[:, ds(toff * 128, 4 * 128)])

        def one_step(h, k, trow):
            """h: half (0/1), k: step-in-half (0..3), trow: runtime step idx."""
            xt = xts[h][:, k * BL : (k + 1) * BL]
            # broadcast dt row -> [128, 128] in PSUM via K=1 matmul with ones
            dtt = dtpool.tile([128, 128], f32, tag="dtt")
            nc.tensor.matmul(
                dtt[:],
                ones[:],
                srows[h][:, k * 128 : (k + 1) * 128],
                start=True,
                stop=True,
            )
            for m in range(ODE_STEPS):
                bread = base[m % 2]
                bwrite = base[(m + 1) % 2]
                ps = pspool.tile([128, 128], f32, tag="pre")
                # ---- base group: pre = Wh.T @ h + Wx.T @ x_aug (per gate cols)
                for g in range(4):
                    nc.tensor.matmul(
                        ps[:, g * BL : (g + 1) * BL],
                        Wh[:, g * 128 : (g + 1) * 128],
                        bread[:, BL : 2 * BL],
                        start=(g == 0),
                        stop=True,
                        skip_group_check=True,
                    )
                    nc.tensor.matmul(
                        ps[:, g * BL : (g + 1) * BL],
                        Wx[:, g * 128 : (g + 1) * 128],
                        xt,
                        start=False,
                        stop=True,
                        skip_group_check=True,
                    )
                for j in range(4):
                    if j == 0:
                        s = bread
                    else:
                        s = stile[(j + 1) % 2]
                        # stage matmul: pre += Wh.T @ (kd_{j-1} - kd_{j-2})_h
                        if j == 1:
                            rhs = kd4[:, BL : 2 * BL, 0]
                        else:
                            rhs = work.tile([128, BL], f32, tag="mmrhs")
                            nc.vector.tensor_tensor(
                                rhs[:],
                                kd4[:, BL : 2 * BL, j - 1],
                                kd4[:, BL : 2 * BL, j - 2],
                                Alu.subtract,
                            )
                            rhs = rhs[:]
                        for g in range(4):
                            nc.tensor.matmul(
                                ps[:, g * BL : (g + 1) * BL],
                                Wh[:, g * 128 : (g + 1) * 128],
                                rhs,
                                start=False,
                                stop=True,
                                skip_group_check=True,
                            )
                    # ---- elementwise stage
                    T = work.tile([128, 5 * BL], f32, tag="T")
                    nc.scalar.activation(T[:, 0 : 4 * BL], ps[:, :], Act.Tanh)
                    nc.scalar.activation(
                        T[:, 4 * BL : 5 * BL], s[:, 0:BL], Act.Tanh
                    )
                    P = work.tile([128, 2 * BL], f32, tag="P")
                    # P = (T[i,o] + 1) * [Tg, tanh(c)] = [2ig | 2o*tanh(c)]
                    nc.vector.scalar_tensor_tensor(
                        P[:], T[:, BL : 3 * BL], 1.0, T[:, 3 * BL : 5 * BL],
                        Alu.add, Alu.mult,
                    )
                    Fq = work.tile([128, BL], f32, tag="Fq")
                    # Fq = (Tf - 1) * c = 2(f-1)c
                    nc.vector.scalar_tensor_tensor(
                        Fq[:], T[:, 0:BL], 1.0, s[:, 0:BL], Alu.subtract, Alu.mult
                    )
                    k2 = work.tile([128, 2 * BL], f32, tag="k2")
                    nc.vector.tensor_tensor(k2[:, 0:BL], P[:, 0:BL], Fq[:], Alu.add)
                    # k2h = -2*h + 2*o*tanh(c)
                    nc.vector.scalar_tensor_tensor(
                        k2[:, BL : 2 * BL], s[:, BL : 2 * BL], -2.0,
                        P[:, BL : 2 * BL], Alu.mult, Alu.add,
                    )
                    # kd_j = dt_j * k2  (dtt cols: 0:64 = dt/16, 64:128 = dt/8)
                    dslice = dtt[:, 0 : 2 * BL] if j < 2 else dtt[:, 2 * BL : 4 * BL]
                    nc.vector.tensor_tensor(kd4[:, :, j], k2[:], dslice, Alu.mult)
                    if j < 3:
                        nc.vector.tensor_tensor(
                            stile[j % 2][:], bread[:], kd4[:, :, j], Alu.add
                        )
                # ---- RK4 combine: scan gives S = 2kd0+4kd1+2kd2+kd3 at j=3 cols
                sc = work.tile([128, 8 * BL], f32, tag="sc")
                nc.vector.tensor_tensor_scan(
                    sc[:], swts[:], kdall[:], 0.0, Alu.mult, Alu.add
                )
                nc.vector.scalar_tensor_tensor(
                    bwrite[:],
                    sc[:].rearrange("p (n j) -> p n j", j=4)[:, :, 3],
                    1.0 / 6.0,
                    bread[:],
                    Alu.mult,
                    Alu.add,
                )
            # write h half of the final state for this step (bf16 downcast)
            hb = work.tile([128, BL], bf16, tag="hb")
            nc.vector.tensor_copy(hb[:], base[0][:, BL : 2 * BL])
            if isinstance(trow, int):
                nc.sync.dma_start(
                    out_d[trow * 128 : (trow + 1) * 128, :], hb[:]
                )
            else:
                nc.sync.dma_start(out_d[ds(trow * 128, 128), :], hb[:])

        # prologue: half 0 <- steps 0..3
        load_half(0, 0)

        if n_steps <= NSLOT:
            # static tiny version (for simulation/debug)
            load_half(1, 4)
            for t in range(n_steps):
                one_step(t // 4, t % 4, t)
        else:
            assert n_steps % NSLOT == 0
            with tc.For_i(0, n_steps, NSLOT) as i:
                load_half(1, i + 4)
                for k in range(4):
                    one_step(0, k, i + k)
                load_half(0, i + NSLOT)
                for k in range(4):
                    one_step(1, k, i + 4 + k)
    nc.finalize()
    return nc


_NC_CACHE = {}


def _get_nc(n_steps=S):
    if n_steps not in _NC_CACHE:
        import concourse.bacc as bacc

        nc = bacc.Bacc(
            "TRN2", target_bir_lowering=False, debug=False, num_devices=NCORES
        )
        _NC_CACHE[n_steps] = _build(nc, n_steps)
    return _NC_CACHE[n_steps]


# ---------------------------------------------------------------------------
# Custom exec path: like bass2jax.run_bass_via_pjrt but with a cached jit and
# device-resident zero output buffers (no 67MB host->device zeros per call).
# ---------------------------------------------------------------------------

_FN_CACHE = {}


def _get_runner(nc):
    key = id(nc)
    if key in _FN_CACHE:
        return _FN_CACHE[key]

    import jax
    import jax.numpy as jnp
    from jax.sharding import Mesh, PartitionSpec, NamedSharding
    from jax.experimental.shard_map import shard_map
    import concourse.mybir as mybir
    from concourse.bass2jax import (
        _bass_exec_p,
        install_neuronx_cc_hook,
        partition_id_tensor,
    )

    install_neuronx_cc_hook()

    partition_name = (
        nc.partition_id_tensor.name if nc.partition_id_tensor else None
    )
    in_names, out_names, out_avals = [], [], []
    for alloc in nc.m.functions[0].allocations:
        if not isinstance(alloc, mybir.MemoryLocationSet):
            continue
        name = alloc.memorylocations[0].name
        if alloc.kind == "ExternalInput":
            if name != partition_name:
                in_names.append(name)
        elif alloc.kind == "ExternalOutput":
            shape = tuple(alloc.tensor_shape)
            dtype = mybir.dt.np(alloc.dtype)
            out_names.append(name)
            out_avals.append(jax.core.ShapedArray(shape, dtype))
    n_params = len(in_names)
    all_names = in_names + out_names
    if partition_name is not None:
        all_names = all_names + [partition_name]

    def _body(*args):
        operands = list(args)
        operands.append(partition_id_tensor())
        outs = _bass_exec_p.bind(
            *operands,
            out_avals=tuple(out_avals),
            in_names=tuple(all_names),
            out_names=tuple(out_names),
            lowering_input_output_aliases=(),
            sim_require_finite=True,
            sim_require_nnan=True,
            nc=nc,
        )
        return tuple(outs)

    devices = jax.devices()[:NCORES]
    mesh = Mesh(np.asarray(devices), ("core",))
    nin = n_params + len(out_names)
    fn = jax.jit(
        shard_map(
            _body,
            mesh=mesh,
            in_specs=(PartitionSpec("core"),) * nin,
            out_specs=(PartitionSpec("core"),) * len(out_names),
            check_rep=False,
        ),
        keep_unused=True,
    )
    sharding = NamedSharding(mesh, PartitionSpec("core"))
    # device-resident zero stand-ins for the output buffers (never donated,
    # never mutated -- the kernel writes every output element)
    zeros = [
        jax.jit(
            lambda a=a: jnp.zeros((NCORES * a.shape[0], *a.shape[1:]), a.dtype),
            out_shardings=sharding,
        )()
        for a in out_avals
    ]
    runner = (fn, in_names, out_names, out_avals, zeros, sharding)
    _FN_CACHE[key] = runner
    return runner


class _Results:
    """Minimal stand-in for BassKernelResults (test.py reads .results/.exec_time_ns)."""

    def __init__(self, results):
        self.results = results
        self.exec_time_ns = None


def _run(nc, in_maps):
    fn, in_names, out_names, out_avals, zeros, _ = _get_runner(nc)
    concat_in = [
        np.concatenate([in_maps[c][n] for c in range(NCORES)], axis=0)
        for n in in_names
    ]
    out_arrs = fn(*concat_in, *zeros)
    results = [
        {
            name: np.asarray(out_arrs[i]).reshape(NCORES, *out_avals[i].shape)[c]
            for i, name in enumerate(out_names)
        }
        for c in range(NCORES)
    ]
    return _Results(results)


def kernel(x, time_diffs, W_i, b_i, W_f, b_f, W_o, b_o, W_g, b_g):
    x = np.asarray(x, np.float32)
    time_diffs = np.asarray(time_diffs, np.float32)
    Ws = {"i": W_i, "f": W_f, "o": W_o, "g": W_g}
    bs = {"i": b_i, "f": b_f, "o": b_o, "g": b_g}
    Ws = {k: np.asarray(v, np.float32) for k, v in Ws.items()}
    bs = {k: np.asarray(v, np.float32) for k, v in bs.items()}

    in_maps = _host_prep(x, time_diffs, Ws, bs)
    nc = _get_nc(S)
    res = _run(nc, in_maps)
    globals()["_last_results"] = res
    out = np.empty((B, S, H), np.float32)
    for c in range(NCORES):
        hT = (
            res.results[c]["hT"].astype(np.float32).reshape(S, 128, BL)
        )  # [t, h, b]
        out[BL * c : BL * (c + 1)] = hT.transpose(2, 0, 1)
    return out


def _bench_device(iters=3):
    """Time the jitted exec with pre-staged device inputs (upload excluded)."""
    import time
    import jax

    names = ["x", "time_diffs"] + [
        f"{p}_{g}" for g in "ifog" for p in ("W", "b")
    ]
    ins = {n: np.load(f"/root/problem/work/in_{n}.npy") for n in names}
    Ws = {k: ins[f"W_{k}"] for k in "ifog"}
    bs = {k: ins[f"b_{k}"] for k in "ifog"}
    in_maps = _host_prep(ins["x"], ins["time_diffs"], Ws, bs)
    nc = _get_nc(S)
    fn, in_names, out_names, out_avals, zeros, sharding = _get_runner(nc)
    concat_in = [
        np.concatenate([in_maps[c][n] for c in range(NCORES)], axis=0)
        for n in in_names
    ]
    dev_in = [jax.device_put(a, sharding) for a in concat_in]
    for a in dev_in:
        a.block_until_ready()
    outs = fn(*dev_in, *zeros)  # warm (compile already cached)
    [o.block_until_ready() for o in outs]
    times = []
    for _ in range(iters):
        t0 = time.time()
        outs = fn(*dev_in, *zeros)
        [o.block_until_ready() for o in outs]
        times.append(time.time() - t0)
    return min(times)


if __name__ == "__main__":
    # quick build-only check
    n = int(sys.argv[1]) if len(sys.argv) > 1 else 8
    nc = _get_nc(n)
    print(
        "built ok, instructions:",
        sum(len(bb.instructions) for bb in nc.m.functions[0].blocks),
    )
